# revision 1
# baseline (speedup 1.0000x reference)
"""CausalBiTrilinearBCNAttention Trainium2 kernel.

Math refactorization: every use of Q, K, invQ, invK in the reference is
through a rank-R projection, and causal cumsum commutes with right
multiplication, so the network collapses to

    xp  = x @ P                  P = [a1|a2s|a3|b1|b2|b3|b7]  (D x 448)
    cumc = causal_cumsum(xp[..., 192:448]) / counts
    g1  = xp[:,0:64]*cumc[:,0:64] + xp[:,64:128]*cumc[:,64:128]
    g2  = xp[:,128:192]*cumc[:,128:192] * cumc[:,192:256]
    out = [g1|g2] @ A.T          A = [WO@U_b | alpha_tri*WO@U_t]  (D x 128)

with  a1=WQ^T V_b, a2s=alpha_bi WQ^T Winv^T W_b, a3=WQ^T V_t,
      b1=WK^T W_b, b2=WK^T Winv^T V_b, b3=WK^T W_t, b7=X_t.

Sharding: 8 cores = 4 batches x 2 T-halves; the T/2 cumsum carry for the
second half is rebuilt on device from sx = sum_t x[b,:T/2] (host computes
only the data reduction; sx @ P happens on device).

Device dataflow, all [token-partition, feature-free], fp16 operands with
fp32 PSUM accumulation (~5e-4 relative error end to end):
  - xp matmuls stream dk-chunks right behind the interleaved xT/P DMAs
    so TensorE starts early and stays HAM-warm.
  - per-128-token-tile cumsum = U^T @ xp_tile on PE (U = upper-tri ones),
    carry broadcast added in the same PSUM group via Ek^T @ cumc_{k-1},
    Ek one-hot row 127 scaled by counts[last token of k-1] (cumc is
    stored pre-divided by counts; counts <= 2048 are exact in fp16).
  - G transposed 128x128 on PE, final [T,128]@[128,D] matmul in fp16.
"""

import numpy as np

import concourse.bass as bass
import concourse.tile as tile
from concourse import bacc, mybir
from concourse.bass_utils import run_bass_kernel_spmd

B, T, D, R = 4, 2048, 1024, 64
TH = T // 2          # tokens per core
NT = TH // 128       # 8 token tiles per core
ND = D // 128        # 8 d chunks
PCOLS = 448          # 7 * R
CUM0 = 192           # start of cumsum group in P's columns
NCUM = 256           # cumsum group width

F32 = mybir.dt.float32
F16 = mybir.dt.float16


def build_nc():
    nc = bacc.Bacc(None, target_bir_lowering=False)

    xT = nc.dram_tensor("xT", [D, TH], F16, kind="ExternalInput")
    P = nc.dram_tensor("P", [D, PCOLS], F16, kind="ExternalInput")
    AT = nc.dram_tensor("AT", [128, D], F16, kind="ExternalInput")
    sxT = nc.dram_tensor("sxT", [128, ND], F16, kind="ExternalInput")
    invc = nc.dram_tensor("invc", [128, NT], F32, kind="ExternalInput")
    outT = nc.dram_tensor("outT", [D, TH], F16, kind="ExternalOutput")

    from concourse.masks import make_identity, make_upper_triangular

    with tile.TileContext(nc) as tc:
        with tc.tile_pool(name="consts", bufs=1) as consts, \
             tc.tile_pool(name="big", bufs=1) as big, \
             tc.tile_pool(name="outp", bufs=4) as outp, \
             tc.tile_pool(name="ps", bufs=2, space="PSUM") as ps:

            # ---- PE warmup: dependency-free matmul burst so the HAM
            # un-throttles (K=8/8) before the real stream arrives ----
            warm_sb = consts.tile([128, 128], F16)
            nc.vector.memset(warm_sb, 0.0)
            warm_ps = ps.tile([128, 512], F32, tag="work", bufs=6)
            for i in range(10):
                nc.tensor.matmul(warm_ps[:, 0:128], warm_sb, warm_sb,
                                 start=True, stop=True)

            # ---- constants built on idle engines (no DMA) ----
            U_sb = consts.tile([128, 128], F16)
            make_upper_triangular(nc, U_sb, val=1.0, diag=True)
            IDN_sb = consts.tile([128, 128], F16)
            make_identity(nc, IDN_sb)
            ones_sb = consts.tile([1, 128], F16)
            nc.vector.memset(ones_sb, 1.0)
            onescol_sb = consts.tile([128, 1], F16)
            nc.vector.memset(onescol_sb, 1.0)

            # ---- loads interleaved across both HWDGE queues so the
            # dk-chunk pairs arrive in consumption order ----
            xT_sb = big.tile([128, ND, TH], F16)
            P_sb = consts.tile([128, ND, PCOLS], F16)
            sxT_sb = consts.tile([128, ND], F16)
            invc_sb = consts.tile([128, NT], F32)
            AT_sb = consts.tile([128, D], F16)
            xTv = xT.rearrange("(k p) t -> p k t", p=128)
            Pv = P.rearrange("(k p) c -> p k c", p=128)
            for j in range(ND // 2):
                qx = nc.sync if j % 2 == 0 else nc.scalar
                qp = nc.scalar if j % 2 == 0 else nc.sync
                qx.dma_start(out=xT_sb[:, 2 * j:2 * j + 2, :],
                             in_=xTv[:, 2 * j:2 * j + 2, :])
                qp.dma_start(out=P_sb[:, 2 * j:2 * j + 2, :],
                             in_=Pv[:, 2 * j:2 * j + 2, :])
            nc.scalar.dma_start(out=sxT_sb, in_=sxT[:, :])
            nc.scalar.dma_start(out=invc_sb, in_=invc[:, :])
            nc.scalar.dma_start(out=AT_sb, in_=AT[:, :])

            xp_sb = big.tile([128, NT, PCOLS], F16)
            cum_sb = big.tile([128, NT, NCUM], F16)
            carry_sb = big.tile([1, NT, NCUM], F16)

            # ---- xp phase A: tiles 0..4 accumulate chunk-by-chunk right
            # behind the DMA stream (5 MMs/chunk ~ matches arrival rate) ----
            NA = 6
            gx = [ps.tile([128, PCOLS], F32, tag="work", bufs=6,
                          name=f"gx_{i}") for i in range(NA)]
            for dk in range(ND):
                for k in range(NA):
                    nc.tensor.matmul(gx[k],
                                     xT_sb[:, dk, k * 128:(k + 1) * 128],
                                     P_sb[:, dk, :],
                                     start=(dk == 0), stop=(dk == ND - 1))
            for k in range(NA):
                nc.vector.tensor_copy(xp_sb[:, k, CUM0:], gx[k][:, CUM0:])
                nc.scalar.copy(xp_sb[:, k, :CUM0], gx[k][:, :CUM0])

            def emit_xp(k):
                xp_ps = ps.tile([128, PCOLS], F32, tag="work", bufs=6)
                for dk in range(ND):
                    nc.tensor.matmul(xp_ps,
                                     xT_sb[:, dk, k * 128:(k + 1) * 128],
                                     P_sb[:, dk, :],
                                     start=(dk == 0), stop=(dk == ND - 1))
                nc.vector.tensor_copy(xp_sb[:, k, CUM0:], xp_ps[:, CUM0:])
                nc.scalar.copy(xp_sb[:, k, :CUM0], xp_ps[:, :CUM0])

            # carry-init: running total seeded with sx @ P[:, CUM0:]
            tot_ps = ps.tile([1, NCUM], F32, tag="tot", bufs=1)
            for dk in range(ND):
                nc.tensor.matmul(tot_ps, sxT_sb[:, dk:dk + 1],
                                 P_sb[:, dk, CUM0:],
                                 start=(dk == 0), stop=(dk == ND - 1))

            def emit_cum(k):
                # snapshot the carry (= totals of everything before tile k)
                nc.vector.tensor_copy(carry_sb[:, k, :], tot_ps[0:1, :])
                # append tile k's local column-total to the running total
                nc.tensor.matmul(tot_ps, onescol_sb, xp_sb[:, k, CUM0:],
                                 start=False, stop=True)
                # block cumsum + carry broadcast, then normalize by counts
                cum_ps = ps.tile([128, NCUM], F32, tag="cum", bufs=1)
                nc.tensor.matmul(cum_ps, U_sb, xp_sb[:, k, CUM0:],
                                 start=True, stop=False)
                nc.tensor.matmul(cum_ps, ones_sb, carry_sb[:, k, :],
                                 start=False, stop=True)
                nc.scalar.activation(cum_sb[:, k, :], cum_ps,
                                     mybir.ActivationFunctionType.Copy,
                                     scale=invc_sb[:, k:k + 1])

            # elementwise (tiles lo..hi): G = [g1|g2]
            G_sb = big.tile([128, NT, 128], F16)
            m2_sb = big.tile([128, NT, 64], F32)

            def emit_ew(lo, hi):
                xps = xp_sb[:, lo:hi, :]
                cms = cum_sb[:, lo:hi, :]
                m2 = m2_sb[:, lo:hi, :]
                g = G_sb[:, lo:hi, :]
                nc.vector.tensor_mul(m2, xps[:, :, 64:128], cms[:, :, 64:128])
                nc.vector.tensor_mul(g[:, :, 0:64], xps[:, :, 0:64],
                                     cms[:, :, 0:64])
                nc.vector.tensor_add(g[:, :, 0:64], g[:, :, 0:64], m2)
                nc.vector.tensor_mul(g[:, :, 64:128], xps[:, :, 128:192],
                                     cms[:, :, 128:192])
                nc.vector.tensor_mul(g[:, :, 64:128], g[:, :, 64:128],
                                     cms[:, :, 192:256])

            GT_sb = big.tile([128, TH], F16)

            def emit_tp(k):
                gt_ps = ps.tile([128, 128], F16, tag="work", bufs=6)
                nc.tensor.transpose(gt_ps, G_sb[:, k, :], IDN_sb)
                nc.vector.tensor_copy(GT_sb[:, k * 128:(k + 1) * 128], gt_ps)

            def emit_final(n):
                for dk in range(ND):
                    o_ps = ps.tile([128, 512], F32, tag="work", bufs=6)
                    nc.tensor.matmul(o_ps,
                                     AT_sb[:, dk * 128:(dk + 1) * 128],
                                     GT_sb[:, n * 512:(n + 1) * 512],
                                     start=True, stop=True)
                    o_sb = outp.tile([128, 512], F16)
                    nc.vector.tensor_copy(o_sb[:, 0:256], o_ps[:, 0:256])
                    nc.scalar.copy(o_sb[:, 256:512], o_ps[:, 256:512])
                    qo = nc.sync if dk % 2 == 0 else nc.scalar
                    qo.dma_start(
                        out=outT[dk * 128:(dk + 1) * 128,
                                 n * 512:(n + 1) * 512],
                        in_=o_sb)

            emit_xp(6)
            emit_cum(0)
            emit_cum(1)
            emit_xp(7)
            emit_cum(2)
            emit_cum(3)
            emit_ew(0, 4)
            for k in range(4):
                emit_tp(k)
            emit_final(0)
            emit_cum(4)
            emit_cum(5)
            emit_cum(6)
            emit_cum(7)
            emit_ew(4, NT)
            for k in range(4, NT):
                emit_tp(k)
            emit_final(1)

    nc.finalize()
    return nc


_NC = None


def _get_nc():
    global _NC
    if _NC is None:
        _NC = build_nc()
    return _NC


def _fold_weights(WQ, WK, WO, Winv, U_b, V_b, W_b, U_t, V_t, W_t, X_t,
                  alpha_bi, alpha_tri):
    f8 = np.float64
    WQ, WK, WO, Winv = (np.asarray(m) for m in (WQ, WK, WO, Winv))
    U_b, V_b, W_b = (np.asarray(m) for m in (U_b, V_b, W_b))
    U_t, V_t, W_t, X_t = (np.asarray(m) for m in (U_t, V_t, W_t, X_t))
    WQt = WQ.astype(f8).T
    WKt = WK.astype(f8).T
    Winvt = Winv.astype(f8).T
    P = np.concatenate([
        WQt @ V_b.astype(f8),
        float(alpha_bi) * (WQt @ (Winvt @ W_b.astype(f8))),
        WQt @ V_t.astype(f8),
        WKt @ W_b.astype(f8),
        WKt @ (Winvt @ V_b.astype(f8)),
        WKt @ W_t.astype(f8),
        X_t.astype(f8),
    ], axis=1).astype(np.float32)
    A = np.concatenate([
        WO.astype(f8) @ U_b.astype(f8),
        float(alpha_tri) * (WO.astype(f8) @ U_t.astype(f8)),
    ], axis=1).astype(np.float32)
    return P, A


def _make_consts(h):
    counts = np.arange(h * TH + 1, (h + 1) * TH + 1, dtype=np.float64)
    invc = np.ascontiguousarray(
        (1.0 / counts).astype(np.float32).reshape(NT, 128).T)
    return invc


def make_in_maps(x, P, A):
    AT = np.ascontiguousarray(A.T.astype(np.float16))
    P16 = P.astype(np.float16)
    in_maps = []
    for core in range(8):
        b, h = core // 2, core % 2
        xTc = np.ascontiguousarray(x[b, h * TH:(h + 1) * TH, :].T
                                   .astype(np.float16))
        if h == 1:
            sx = x[b, :TH, :].sum(axis=0, dtype=np.float64)
        else:
            sx = np.zeros(D, np.float64)
        sxT = np.ascontiguousarray(
            sx.astype(np.float16).reshape(ND, 128).T)
        invc = _make_consts(h)
        in_maps.append(dict(xT=xTc, P=P16, AT=AT, sxT=sxT, invc=invc))
    return in_maps


def kernel(x, WQ, WK, WO, Winv, U_b, V_b, W_b, bias_b,
           U_t, V_t, W_t, X_t, bias_t, alpha_bi, alpha_tri):
    x = np.asarray(x, dtype=np.float32)
    P, A = _fold_weights(WQ, WK, WO, Winv, U_b, V_b, W_b,
                         U_t, V_t, W_t, X_t, alpha_bi, alpha_tri)
    in_maps = make_in_maps(x, P, A)

    res = run_bass_kernel_spmd(_get_nc(), in_maps, core_ids=list(range(8)))

    out = np.empty((B, T, D), np.float32)
    for core in range(8):
        b, h = core // 2, core % 2
        out[b, h * TH:(h + 1) * TH, :] = \
            res.results[core]["outT"].T.astype(np.float32)

    # constant bias term (zero for the given inputs, kept for fidelity)
    bias_out = ((1.0 + float(alpha_bi)) * np.asarray(bias_b, np.float64)
                + float(alpha_tri) * np.asarray(bias_t, np.float64)) \
        @ np.asarray(WO, np.float64).T
    if np.any(bias_out):
        out += bias_out.astype(np.float32)[None, None, :]
    return out



# revision 7
# speedup vs baseline: 1.0697x; 1.0697x over previous
"""CausalBiTrilinearBCNAttention Trainium2 kernel.

Math refactorization: every use of Q, K, invQ, invK in the reference is
through a rank-R projection, and causal cumsum commutes with right
multiplication, so the network collapses to

    xp  = x @ P                  P = [a1|a2s|a3|b1|b2|b3|b7]  (D x 448)
    cumc = causal_cumsum(xp[..., 192:448]) / counts
    g1  = xp[:,0:64]*cumc[:,0:64] + xp[:,64:128]*cumc[:,64:128]
    g2  = xp[:,128:192]*cumc[:,128:192] * cumc[:,192:256]
    out = [g1|g2] @ A.T          A = [WO@U_b | alpha_tri*WO@U_t]  (D x 128)

with  a1=WQ^T V_b, a2s=alpha_bi WQ^T Winv^T W_b, a3=WQ^T V_t,
      b1=WK^T W_b, b2=WK^T Winv^T V_b, b3=WK^T W_t, b7=X_t.

Sharding: 8 cores = 4 batches x 2 T-halves.  The per-128-token-tile
cumsum carries (sums of all earlier tokens' xp rows, including the
other half's for h=1) are computed on host -- an O(B*T*D) reduction
plus an O(B*NT*D*R) mini-matmul, ~100x smaller than the device work --
and shipped as a [1, NT*256] input, which removes the serial on-device
carry chain entirely.

Device schedule (all [token-partition, feature-free], fp16 operands,
fp32 PSUM):
  - input DMAs are issued before anything else (x chunks alternate the
    sync/scalar queues, P + small tensors stream on the gpsimd queue)
    so HBM transfers start the moment the framework preamble ends.
  - PE warmup burst while DMA streams (HAM un-throttle).
  - xp phase A: tiles 0..5 accumulate chunk-by-chunk behind the DMA
    stream in 6 PSUM banks; tiles 6,7 re-read SBUF afterwards.
  - tile-pair cumsum: one U^T matmul per 2 tiles (N=512) plus a K=1
    carry-broadcast matmul into the same PSUM group; normalization by
    1/counts happens in the scalar-engine PSUM->SBUF activation copy.
  - cum / transpose / final matmuls are interleaved into the PE queue
    so it never idles (keeps HAM at K=8/8); PSUM is managed as a
    single 8-slot ring.
  - output: final [T,128]@[128,D] matmuls per 512-token slab, PSUM->
    SBUF copies split across scalar+gpsimd, DMA spread over 4 queues.
"""

import numpy as np

import concourse.bass as bass
import concourse.tile as tile
from concourse import bacc, mybir
from concourse.bass_utils import run_bass_kernel_spmd

B, T, D, R = 4, 2048, 1024, 64
TH = T // 2          # tokens per core
NT = TH // 128       # 8 token tiles per core
ND = D // 128        # 8 d chunks
PCOLS = 448          # 7 * R
CUM0 = 192           # start of cumsum group in P's columns
NCUM = 256           # cumsum group width

F32 = mybir.dt.float32
F16 = mybir.dt.float16


def build_nc():
    nc = bacc.Bacc(None, target_bir_lowering=False)

    xT = nc.dram_tensor("xT", [D, TH], F16, kind="ExternalInput")
    P = nc.dram_tensor("P", [D, PCOLS], F16, kind="ExternalInput")
    AT = nc.dram_tensor("AT", [128, D], F16, kind="ExternalInput")
    invc = nc.dram_tensor("invc", [128, NT], F32, kind="ExternalInput")
    carry = nc.dram_tensor("carry", [1, NT * NCUM], F16, kind="ExternalInput")
    outT = nc.dram_tensor("outT", [D, TH], F16, kind="ExternalOutput")

    from concourse.masks import make_identity, make_upper_triangular

    with tile.TileContext(nc) as tc:
        with tc.tile_pool(name="consts", bufs=1) as consts, \
             tc.tile_pool(name="big", bufs=1) as big, \
             tc.tile_pool(name="outp", bufs=8) as outp, \
             tc.tile_pool(name="ps", bufs=8, space="PSUM") as ps:

            # ---- input DMAs first: sync/scalar stream the x chunks,
            # gpsimd streams P + the small tensors ----
            xT_sb = big.tile([128, ND, TH], F16)
            P_sb = consts.tile([128, ND, PCOLS], F16)
            AT_sb = consts.tile([128, D], F16)
            invc_sb = consts.tile([128, NT], F32)
            carry_sb = consts.tile([1, NT, NCUM], F16)
            xTv = xT.rearrange("(k p) t -> p k t", p=128)
            Pv = P.rearrange("(k p) c -> p k c", p=128)
            for dk in range(ND):
                q = nc.sync if dk % 2 == 0 else nc.scalar
                q.dma_start(out=xT_sb[:, dk, :], in_=xTv[:, dk, :])
            nc.gpsimd.dma_start(out=P_sb[:, 0:2, :], in_=Pv[:, 0:2, :])
            nc.gpsimd.dma_start(out=invc_sb, in_=invc[:, :])
            nc.gpsimd.dma_start(
                out=carry_sb,
                in_=carry.rearrange("o (k c) -> o k c", k=NT))
            nc.gpsimd.dma_start(out=P_sb[:, 2:5, :], in_=Pv[:, 2:5, :])
            nc.gpsimd.dma_start(out=P_sb[:, 5:8, :], in_=Pv[:, 5:8, :])
            nc.gpsimd.dma_start(out=AT_sb, in_=AT[:, :])

            # ---- constants on otherwise-idle engines ----
            warm_sb = consts.tile([128, 128], F16)
            nc.vector.memset(warm_sb, 0.0)
            ones_sb = consts.tile([1, 128], F16)
            nc.vector.memset(ones_sb, 1.0)
            U_sb = consts.tile([128, 128], F16)
            make_upper_triangular(nc, U_sb, val=1.0, diag=True)
            IDN_sb = consts.tile([128, 128], F16)
            make_identity(nc, IDN_sb)

            # ---- PE warmup: dependency-free burst so the HAM
            # un-throttles before the real stream arrives ----
            warm_ps = ps.tile([128, 512], F32, tag="bank")
            for _ in range(12):
                nc.tensor.matmul(warm_ps[:, 0:128], warm_sb, warm_sb,
                                 start=True, stop=True)

            # ---- xp phase A: tiles 0..5 accumulate chunk-by-chunk
            # right behind the DMA stream ----
            NA = 6
            gx = [ps.tile([128, PCOLS], F32, tag="bank", name=f"gx_{i}")
                  for i in range(NA)]
            for dk in range(ND):
                for k in range(NA):
                    nc.tensor.matmul(gx[k],
                                     xT_sb[:, dk, k * 128:(k + 1) * 128],
                                     P_sb[:, dk, :],
                                     start=(dk == 0), stop=(dk == ND - 1))

            xp6_ps = ps.tile([128, PCOLS], F32, tag="bank")
            for dk in range(ND):
                nc.tensor.matmul(xp6_ps,
                                 xT_sb[:, dk, 6 * 128:7 * 128],
                                 P_sb[:, dk, :],
                                 start=(dk == 0), stop=(dk == ND - 1))

            xp_sb = big.tile([128, NT, PCOLS], F16)

            def copy_xp(k, src):
                # vector: cumsum columns (feeds the U matmul);
                # scalar: the elementwise-only columns
                nc.vector.tensor_copy(xp_sb[:, k, CUM0:], src[:, CUM0:])
                nc.scalar.copy(xp_sb[:, k, :CUM0], src[:, :CUM0])

            for k in range(NA):
                copy_xp(k, gx[k])

            xp7_ps = ps.tile([128, PCOLS], F32, tag="bank")
            for dk in range(ND):
                nc.tensor.matmul(xp7_ps,
                                 xT_sb[:, dk, 7 * 128:8 * 128],
                                 P_sb[:, dk, :],
                                 start=(dk == 0), stop=(dk == ND - 1))
            copy_xp(6, xp6_ps)
            copy_xp(7, xp7_ps)

            # ---- cumsum per tile pair: cum = U^T @ xp + 1 (x) carry,
            # then normalize by 1/counts in the scalar copy ----
            cum_sb = big.tile([128, NT, NCUM], F16)
            cum_ps = []

            def emit_cum_pair(j):
                k = 2 * j
                c_ps = ps.tile([128, 2, NCUM], F32, tag="bank",
                               name=f"cum_{j}")
                nc.tensor.matmul(c_ps, U_sb, xp_sb[:, k:k + 2, CUM0:],
                                 start=True, stop=False)
                nc.tensor.matmul(c_ps, ones_sb, carry_sb[:, k:k + 2, :],
                                 start=False, stop=True)
                cum_ps.append(c_ps)
                for i in (0, 1):
                    nc.scalar.activation(
                        cum_sb[:, k + i, :], c_ps[:, i, :],
                        mybir.ActivationFunctionType.Copy,
                        scale=invc_sb[:, k + i:k + i + 1])

            # ---- elementwise per tile pair: G = [g1|g2] ----
            G_sb = big.tile([128, NT, 128], F16)
            m2_sb = big.tile([128, 2, 64], F32)
            t2_sb = big.tile([128, 2, 64], F32)

            def emit_ew(j):
                k = 2 * j
                xps = xp_sb[:, k:k + 2, :]
                cms = cum_sb[:, k:k + 2, :]
                g = G_sb[:, k:k + 2, :]
                nc.gpsimd.tensor_mul(m2_sb, xps[:, :, 64:128],
                                     cms[:, :, 64:128])
                nc.gpsimd.tensor_mul(t2_sb, xps[:, :, 0:64],
                                     cms[:, :, 0:64])
                nc.gpsimd.tensor_add(g[:, :, 0:64], t2_sb, m2_sb)
                nc.gpsimd.tensor_mul(g[:, :, 64:128], xps[:, :, 128:192],
                                     cms[:, :, 128:192])
                nc.gpsimd.tensor_mul(g[:, :, 64:128], g[:, :, 64:128],
                                     cms[:, :, 192:256])

            GT_sb = big.tile([128, TH], F16)

            def emit_tp(k):
                gt_ps = ps.tile([128, 128], F16, tag="bank",
                                name=f"gt_{k}")
                nc.tensor.transpose(gt_ps, G_sb[:, k, :], IDN_sb)
                nc.vector.tensor_copy(GT_sb[:, k * 128:(k + 1) * 128], gt_ps)

            def emit_final(n):
                # out copies split scalar/gpsimd; DMA spread on the
                # copier queues + sync so the drain uses 4 queues
                for dk in range(ND):
                    o_ps = ps.tile([128, 512], F32, tag="bank",
                                   name=f"o_{n}_{dk}")
                    nc.tensor.matmul(o_ps,
                                     AT_sb[:, dk * 128:(dk + 1) * 128],
                                     GT_sb[:, n * 512:(n + 1) * 512],
                                     start=True, stop=True)
                    o_sb = outp.tile([128, 512], F16, name=f"ob_{n}_{dk}")
                    nc.vector.tensor_copy(o_sb[:, 0:256], o_ps[:, 0:256])
                    nc.scalar.copy(o_sb[:, 256:512], o_ps[:, 256:512])
                    qo = (nc.sync, nc.scalar, nc.gpsimd)[dk % 3]
                    qo.dma_start(
                        out=outT[dk * 128:(dk + 1) * 128,
                                 n * 512:(n + 1) * 512],
                        in_=o_sb)

            emit_cum_pair(0)
            emit_cum_pair(1)
            emit_cum_pair(2)
            emit_cum_pair(3)
            emit_ew(0)
            emit_ew(1)
            emit_tp(0)
            emit_tp(1)
            emit_tp(2)
            emit_tp(3)
            emit_final(0)
            emit_ew(2)
            emit_ew(3)
            for k in range(4, NT):
                emit_tp(k)
            emit_final(1)

    nc.finalize()
    return nc


_NC = None


def _get_nc():
    global _NC
    if _NC is None:
        _NC = build_nc()
    return _NC


def _fold_weights(WQ, WK, WO, Winv, U_b, V_b, W_b, U_t, V_t, W_t, X_t,
                  alpha_bi, alpha_tri):
    f8 = np.float64
    WQ, WK, WO, Winv = (np.asarray(m) for m in (WQ, WK, WO, Winv))
    U_b, V_b, W_b = (np.asarray(m) for m in (U_b, V_b, W_b))
    U_t, V_t, W_t, X_t = (np.asarray(m) for m in (U_t, V_t, W_t, X_t))
    WQt = WQ.astype(f8).T
    WKt = WK.astype(f8).T
    Winvt = Winv.astype(f8).T
    P = np.concatenate([
        WQt @ V_b.astype(f8),
        float(alpha_bi) * (WQt @ (Winvt @ W_b.astype(f8))),
        WQt @ V_t.astype(f8),
        WKt @ W_b.astype(f8),
        WKt @ (Winvt @ V_b.astype(f8)),
        WKt @ W_t.astype(f8),
        X_t.astype(f8),
    ], axis=1).astype(np.float32)
    A = np.concatenate([
        WO.astype(f8) @ U_b.astype(f8),
        float(alpha_tri) * (WO.astype(f8) @ U_t.astype(f8)),
    ], axis=1).astype(np.float32)
    return P, A


def _make_consts(h):
    counts = np.arange(h * TH + 1, (h + 1) * TH + 1, dtype=np.float64)
    invc = np.ascontiguousarray(
        (1.0 / counts).astype(np.float32).reshape(NT, 128).T)
    return invc


def make_in_maps(x, P, A):
    AT = np.ascontiguousarray(A.T.astype(np.float16))
    P16 = P.astype(np.float16)
    # host-side carries: cumulative sums of xp's cumsum-column rows up
    # to each tile boundary, matching the device's f16-rounded inputs
    Pc = P16.astype(np.float64)[:, CUM0:]
    in_maps = []
    for core in range(8):
        b, h = core // 2, core % 2
        xb16 = x[b].astype(np.float16).astype(np.float64)
        tsum = xb16.reshape(T // 128, 128, D).sum(axis=1)
        pref = np.cumsum(tsum, axis=0) - tsum          # exclusive prefix
        carr = pref[h * NT:(h + 1) * NT] @ Pc          # [NT, 256]
        carry = np.ascontiguousarray(
            carr.astype(np.float16).reshape(1, NT * NCUM))
        xTc = np.ascontiguousarray(x[b, h * TH:(h + 1) * TH, :].T
                                   .astype(np.float16))
        invc = _make_consts(h)
        in_maps.append(dict(xT=xTc, P=P16, AT=AT, invc=invc, carry=carry))
    return in_maps


def kernel(x, WQ, WK, WO, Winv, U_b, V_b, W_b, bias_b,
           U_t, V_t, W_t, X_t, bias_t, alpha_bi, alpha_tri):
    x = np.asarray(x, dtype=np.float32)
    P, A = _fold_weights(WQ, WK, WO, Winv, U_b, V_b, W_b,
                         U_t, V_t, W_t, X_t, alpha_bi, alpha_tri)
    in_maps = make_in_maps(x, P, A)

    res = run_bass_kernel_spmd(_get_nc(), in_maps, core_ids=list(range(8)))

    out = np.empty((B, T, D), np.float32)
    for core in range(8):
        b, h = core // 2, core % 2
        out[b, h * TH:(h + 1) * TH, :] = \
            res.results[core]["outT"].T.astype(np.float32)

    # constant bias term (zero for the given inputs, kept for fidelity)
    bias_out = ((1.0 + float(alpha_bi)) * np.asarray(bias_b, np.float64)
                + float(alpha_tri) * np.asarray(bias_t, np.float64)) \
        @ np.asarray(WO, np.float64).T
    if np.any(bias_out):
        out += bias_out.astype(np.float32)[None, None, :]
    return out
